# revision 32
# baseline (speedup 1.0000x reference)
"""TRN2 Bass kernel for nn_EquivariantConv (GNN message passing), v2.

Strategy (8 NeuronCores, edges partitioned by destination-node block):
- Host does ALL indexing: per-core slot layout (col-sorted, degree-padded,
  packed into 128 partitions), plus value gathers into dense per-core DRAM
  arrays (rowpos/colpos fp32 planes, row features bf16 planes). No indirect
  DMA anywhere on device.
- Device phases per core:
  A) ev = pos[row]-pos[col], r2, r=sqrt(r2) [ACT], rinv=1/r [DVE recip
     approx], u = ev*rinv (bf16). Stages r (fp32) + u (bf16) for all slots.
  B) smooth-finite basis per k: u=Square(r*11/3 - (k+1)) [ACT],
     z=min(u-1,-eps) [GP], p=1/z [DVE approx], emb=C*exp(2p) [ACT] ->
     bf16 emb tile in (slotcol, k16) layout. Only Square/Exp tables.
  C) radial MLP on PE in bf16: one [128,128] fwd transpose per 8 slotcols,
     4x row-tiled (tile_position) mm1 vs block-padded W1, relu copy, 4x
     col-tiled mm2, one [128,128] back transpose per 8 slotcols, strided
     plane-copy of w back to edge-major planes.
  D) tensor product in bf16 planes on DVE/GpSimd, 4-slot partial reduce,
     per-class reduce, dense DRAM write; host unpermutes node sums.
"""

import math
import os
import numpy as np

import concourse.bass as bass
import concourse.bacc as bacc
import concourse.mybir as mybir
from concourse.tile import TileContext
from concourse.bass_utils import run_bass_kernel_spmd

dt = mybir.dt


def _patch_tile_drain():
    """This walrus build rejects drains carrying >1 semaphore wait ("Too many
    sync wait commands"). Split the kernel-tail drain's waits onto separate
    SP drain instructions, one wait each."""
    import concourse.tile as tile_mod
    from concourse.vector_clock import ScopedClock

    if getattr(tile_mod.TileContext, "_drain_patched", False):
        return

    def _drain_and_barrier(self, tick_clock, wait_clock):
        nc = self.nc
        probe = nc.sync.drain()
        wait_clock.add_sem_waits(
            probe.ins, ScopedClock({None: tick_clock.global_clock})
        )
        waits = list(probe.ins.sync_info.on_wait) if probe.ins.sync_info else []
        if len(waits) > 1:
            probe.ins.sync_info.on_wait = waits[:1]
            for w in waits[1:]:
                n2 = nc.sync.drain()
                if n2.ins.sync_info is None:
                    n2.ins.sync_info = mybir.SyncInfo(on_wait=[w], on_update=[])
                else:
                    n2.ins.sync_info.on_wait = [w]
        nc.all_engine_barrier()
        popped = nc._tile_sem_poison_stack.pop()
        assert popped is self._sem_poison
        nc.clear_and_free_semaphores(list(self.sems.allocated().values()))
        nc.all_engine_barrier()

    tile_mod.TileContext._drain_and_barrier = _drain_and_barrier
    tile_mod.TileContext._drain_patched = True


def _install_ntff_shim():
    """Optional: enable NTFF profiling under axon (antenv.axon_hooks shim)."""
    import contextlib
    import ctypes
    import sys
    import types

    if "antenv.axon_hooks" in sys.modules:
        return
    so_path = "/opt/axon/libaxon_pjrt.so"
    if not os.path.exists(so_path):
        return
    try:
        lib = ctypes.CDLL(so_path)
        if not hasattr(lib, "axon_start_nrt_profile"):
            return
        lib.axon_start_nrt_profile.argtypes = [
            ctypes.POINTER(ctypes.c_int64), ctypes.c_size_t]
        lib.axon_start_nrt_profile.restype = ctypes.c_int64
        lib.axon_stop_nrt_profile.argtypes = [ctypes.c_char_p]
        lib.axon_stop_nrt_profile.restype = ctypes.c_int64

        @contextlib.contextmanager
        def _profile(output_dir, device_ids):
            import jax
            jax.devices()
            if device_ids:
                ids = (ctypes.c_int64 * len(device_ids))(*device_ids)
                rc = lib.axon_start_nrt_profile(ids, len(device_ids))
            else:
                rc = lib.axon_start_nrt_profile(None, 0)
            if rc != 0:
                raise RuntimeError(f"axon_start_nrt_profile rc={rc}")
            try:
                yield
            finally:
                lib.axon_stop_nrt_profile(output_dir.encode())

        mod = types.ModuleType("antenv.axon_hooks")
        mod.get_axon_ntff_profile_hook = lambda: _profile
        mod.set_axon_ntff_profile_hook = lambda h: None
        import antenv
        antenv.axon_hooks = mod
        sys.modules["antenv.axon_hooks"] = mod
    except Exception:
        pass


_patch_tile_drain()

LAST_EXEC_NS = None
Alu = mybir.AluOpType
Act = mybir.ActivationFunctionType

N_NODES = 50000
N_EDGES = 1600000
NUM_BASIS = 10
HIDDEN = 64
MAX_RADIUS = 3.0
N_CORES = 8
NPC = N_NODES // N_CORES  # dest nodes per core
P = 128
MEGA = 512  # slot columns per outer phase chunk

USE_TILE_POS = os.environ.get("KERNEL_TILE_POS", "1") == "1"
USE_GP = os.environ.get("KERNEL_GP", "1") == "1"
USE_RECIP_APPROX = os.environ.get("KERNEL_RECIP", "1") == "1"
SKIP_B = os.environ.get("KERNEL_SKIP_B", "0") == "1"
SKIP_MLP = os.environ.get("KERNEL_SKIP_MLP", "0") == "1"
SKIP_D = os.environ.get("KERNEL_SKIP_D", "0") == "1"
SKIP_MM = os.environ.get("KERNEL_SKIP_MM", "0") == "1"
MM_SAFE = os.environ.get("KERNEL_MM_SAFE", "0") == "1"
PC_SAFE = os.environ.get("KERNEL_PC_SAFE", "0") == "1"


def _build_layout(edge_index):
    """Host-side index work: per-core slot layout. Values untouched.

    Cross-core class balancing: per-partition class counts n_k are chosen
    globally from suffix maxima of per-core padded-degree histograms.
    Returns (class_list, NN, F, row_slots, col_slot, node_lid).
    """
    row = edge_index[0].astype(np.int64)
    col = edge_index[1].astype(np.int64)
    core = col // NPC

    per_core = []
    for c in range(N_CORES):
        m = core == c
        row_c = row[m]
        col_c = col[m] - c * NPC
        deg = np.bincount(col_c, minlength=NPC)
        order = np.argsort(col_c, kind="stable")
        row_sorted = row_c[order]
        starts = np.zeros(NPC + 1, np.int64)
        np.cumsum(deg, out=starts[1:])
        nz = np.nonzero(deg)[0]
        pdeg = ((deg[nz] + 3) // 4) * 4
        per_core.append((deg, starts, row_sorted, nz, pdeg))

    # global class sizing: S_k = max over cores of #nodes with pdeg >= k
    all_k = sorted({int(v) for (_, _, _, _, pdeg) in per_core for v in pdeg},
                   reverse=True)
    n_k = {}
    cum = 0
    for k in all_k:
        s_k = max(int((pd >= k).sum()) for (_, _, _, _, pd) in per_core)
        need = max((s_k + P - 1) // P, cum)
        n_k[k] = need - cum
        cum = need
    class_list = [(k, n_k[k]) for k in all_k if n_k[k] > 0]
    class_list = class_list[::-1]  # ascending k

    NN = sum(nk for (_, nk) in class_list)
    F = sum(nk * k for (k, nk) in class_list)
    F_pad = (F + P - 1) // P * P

    row_slots = np.full((N_CORES, P, F_pad), N_NODES, np.int64)
    col_slot = np.full((N_CORES, P, F_pad), N_NODES, np.int64)
    node_lid = np.full((N_CORES, P, NN), NPC + 6, np.int32)

    foffs = {}
    noffs = {}
    fo = 0
    no = 0
    for (k, nk) in class_list:
        foffs[k] = fo
        noffs[k] = no
        fo += nk * k
        no += nk

    desc = [k for (k, _) in class_list][::-1]
    for c in range(N_CORES):
        deg, starts, row_sorted, nz, pdeg = per_core[c]
        order = np.argsort(-pdeg, kind="stable")
        nodes_desc = nz[order]
        pos_in_class = 0
        ki = 0
        for n in nodes_desc:
            while pos_in_class >= n_k[desc[ki]] * P:
                ki += 1
                pos_in_class = 0
            k = desc[ki]
            j = pos_in_class
            p = j % P
            jj = j // P
            d = deg[n]
            f0 = foffs[k] + jj * k
            row_slots[c, p, f0:f0 + d] = row_sorted[starts[n]:starts[n + 1]]
            col_slot[c, p, f0:f0 + k] = c * NPC + n
            node_lid[c, p, noffs[k] + jj] = n
            pos_in_class += 1
    return class_list, NN, F_pad, row_slots, col_slot, node_lid


def _build_program(class_list, NN, F):
    nc = bacc.Bacc(None)
    W1 = nc.declare_dram_parameter("W1", [NUM_BASIS, HIDDEN], dt.float32, isOutput=False)
    W2 = nc.declare_dram_parameter("W2", [HIDDEN, 5], dt.float32, isOutput=False)
    rowpos = nc.declare_dram_parameter("rowpos", [P, 3, F], dt.float32, isOutput=False)
    colpos = nc.declare_dram_parameter("colpos", [P, 3, F], dt.float32, isOutput=False)
    rowf = nc.declare_dram_parameter("rowf", [P, 4, F], dt.float32, isOutput=False)
    yout = nc.declare_dram_parameter("yout", [P, NN, 4], dt.float32, isOutput=True)

    C_EMB = 1.14136 * float(np.e) ** 2
    w1_scale = C_EMB / math.sqrt(NUM_BASIS)
    w2_common = math.sqrt(2.0) / math.sqrt(HIDDEN) / math.sqrt(32.0)
    col_scales = [
        math.sqrt(0.5) * w2_common,            # w0 * x0 * y0
        1.0 * w2_common,                       # w1 * x0 * u (sh norm folded)
        (1.0 / math.sqrt(3.0)) * w2_common,    # w2 * xv * y0
        math.sqrt(0.5) * w2_common,            # w3 * dot(xv, u)
        (1.0 / math.sqrt(2.0)) * w2_common,    # w4 * cross(xv, u)
    ]

    n_mega = (F + MEGA - 1) // MEGA
    megas = [(i * MEGA, min(MEGA, F - i * MEGA)) for i in range(n_mega)]
    F4 = F // 4

    with TileContext(nc) as tc:
        with (
            tc.tile_pool(name="persist", bufs=1) as pp,
            tc.tile_pool(name="chunk", bufs=2) as cp,
            tc.tile_pool(name="mlp", bufs=3) as mp,
            tc.tile_pool(name="pst", bufs=2, space="PSUM") as pst,   # transposes
            tc.tile_pool(name="psh", bufs=2, space="PSUM") as psh,   # h
            tc.tile_pool(name="psw", bufs=2, space="PSUM") as psw,   # w
        ):
            # ---- weights prep (one-time, tiny) ----
            w1f = pp.tile([NUM_BASIS, HIDDEN], dt.float32)
            nc.sync.dma_start(out=w1f[:], in_=W1[:])
            w1b = pp.tile([NUM_BASIS, HIDDEN], dt.bfloat16)
            nc.vector.tensor_scalar_mul(w1b[:], w1f[:], w1_scale)
            w1pad = pp.tile([32, P], dt.bfloat16)
            nc.vector.memset(w1pad[:], 0.0)
            for c in range(2):
                nc.sync.dma_start(
                    out=w1pad[16 * c:16 * c + NUM_BASIS,
                              64 * c:64 * c + HIDDEN],
                    in_=w1b[:])
            w2f = pp.tile([HIDDEN, 5], dt.float32)
            nc.sync.dma_start(out=w2f[:], in_=W2[:])
            w2b = pp.tile([HIDDEN, 5], dt.bfloat16)
            for j, s in enumerate(col_scales):
                nc.vector.tensor_scalar_mul(w2b[:, j:j + 1], w2f[:, j:j + 1], s)
            w2stack = pp.tile([P, P], dt.bfloat16)
            nc.vector.memset(w2stack[:], 0.0)
            for g in range(4):
                for c in range(2):
                    for jj in range(5):
                        nc.sync.dma_start(
                            out=w2stack[64 * c:64 * c + HIDDEN,
                                        32 * g + 2 * jj + c:32 * g + 2 * jj + c + 1],
                            in_=w2b[:, jj:jj + 1])
            identb = pp.tile([P, P], dt.bfloat16)
            from concourse.masks import make_identity
            make_identity(nc, identb[:])
            bconst = pp.tile([P, NUM_BASIS], dt.float32)
            for k in range(NUM_BASIS):
                nc.vector.memset(bconst[:, k:k + 1], -(k + 1.0))

            # ---- persistent stages ----
            rowfs = pp.tile([P, 4, F], dt.bfloat16, name="rowfs")
            for comp in range(4):
                rfst = cp.tile([P, F], dt.float32, tag="rfst", name="rfst")
                nc.sync.dma_start(out=rfst[:], in_=rowf[:, comp, :])
                nc.vector.tensor_copy(out=rowfs[:, comp, :], in_=rfst[:])
            rfull = pp.tile([P, F], dt.float32, name="rfull")
            ufull = pp.tile([P, 3, F], dt.bfloat16, name="ufull")
            g8 = [pp.tile([P, F4], dt.float32, tag=f"g8_{i}", name=f"g8_{i}")
                  for i in range(4)]
            embm = [pp.tile([P, 16, MEGA], dt.bfloat16, name=f"embm{i}")
                    for i in range(2)]
            nc.vector.memset(embm[0][:], 0.0)
            nc.vector.memset(embm[1][:], 0.0)
            wsl = [pp.tile([P, 5, MEGA + 8], dt.bfloat16, name=f"wsl{i}")
                   for i in range(2)]

            # ---- phase A body: geometry for one mega ----
            def do_A(off, MS):
                rp = cp.tile([P, 3, MEGA], dt.float32, tag="rp", name="rp")
                cpx = cp.tile([P, 3, MEGA], dt.float32, tag="cpx", name="cpx")
                nc.sync.dma_start(out=rp[:, :, :MS], in_=rowpos[:, :, off:off + MS])
                nc.sync.dma_start(out=cpx[:, :, :MS], in_=colpos[:, :, off:off + MS])

                def T(tag):
                    return cp.tile([P, MEGA], dt.float32, tag=tag, name=tag)

                ev = cp.tile([P, 3, MEGA], dt.float32, tag="ev", name="ev")
                gpe = nc.gpsimd if USE_GP else nc.vector
                for comp, eng in ((0, gpe), (1, gpe), (2, nc.vector)):
                    eng.tensor_tensor(out=ev[:, comp, :MS], in0=rp[:, comp, :MS],
                                      in1=cpx[:, comp, :MS], op=Alu.subtract)
                sq0, sq1, r2 = T("sq0"), T("sq1"), T("r2")
                gpe.tensor_tensor(out=sq0[:, :MS], in0=ev[:, 0, :MS],
                                  in1=ev[:, 0, :MS], op=Alu.mult)
                gpe.tensor_tensor(out=sq1[:, :MS], in0=ev[:, 1, :MS],
                                  in1=ev[:, 1, :MS], op=Alu.mult)
                nc.vector.tensor_tensor(out=r2[:, :MS], in0=ev[:, 2, :MS],
                                        in1=ev[:, 2, :MS], op=Alu.mult)
                nc.vector.tensor_tensor(out=r2[:, :MS], in0=r2[:, :MS],
                                        in1=sq1[:, :MS], op=Alu.add)
                nc.vector.tensor_tensor(out=r2[:, :MS], in0=r2[:, :MS],
                                        in1=sq0[:, :MS], op=Alu.add)
                nc.vector.tensor_scalar_max(r2[:, :MS], r2[:, :MS], 1e-12)
                nc.scalar.sqrt(out=rfull[:, off:off + MS], in_=r2[:, :MS])
                rinv = T("rinv")
                if USE_RECIP_APPROX:
                    nc.vector.reciprocal_approx_fast(out=rinv[:, :MS],
                                                     in_=rfull[:, off:off + MS])
                else:
                    nc.vector.reciprocal(out=rinv[:, :MS],
                                         in_=rfull[:, off:off + MS])
                for comp in range(3):
                    nc.vector.tensor_tensor(out=ufull[:, comp, off:off + MS],
                                            in0=ev[:, comp, :MS],
                                            in1=rinv[:, :MS], op=Alu.mult)

            # ---- phases B/C/D per mega, phase A interleaved one ahead ----
            do_A(*megas[0])
            for mi, (off, MS) in enumerate(megas):
                if mi + 1 < len(megas):
                    do_A(*megas[mi + 1])
                em = embm[mi % 2]
                ws = wsl[mi % 2]
                rsl = rfull[:, off:off + MS]

                # B: basis -> em[:, :MS, 0:10]
                for k in range(NUM_BASIS if not SKIP_B else 0):
                    zk = cp.tile([P, MEGA], dt.float32, tag="zk", name="zk")
                    pk = cp.tile([P, MEGA], dt.float32, tag="pk", name="pk")
                    nc.scalar.activation(out=zk[:, :MS], in_=rsl, func=Act.Square,
                                         bias=bconst[:, k:k + 1], scale=11.0 / 3.0)
                    nc.vector.tensor_scalar(out=zk[:, :MS], in0=zk[:, :MS],
                                            scalar1=1.0, scalar2=-1e-30,
                                            op0=Alu.subtract, op1=Alu.min)
                    if USE_RECIP_APPROX:
                        nc.vector.reciprocal_approx_fast(out=pk[:, :MS],
                                                         in_=zk[:, :MS])
                    else:
                        nc.vector.reciprocal(out=pk[:, :MS], in_=zk[:, :MS])
                    nc.scalar.activation(out=em[:, k, :MS], in_=pk[:, :MS],
                                         func=Act.Exp, bias=0.0, scale=2.0)

                # C: MLP, 32 slotcols per iteration
                if mi == 0 and SKIP_MLP:
                    nc.vector.memset(wsl[0][:], 0.0)
                    nc.vector.memset(wsl[1][:], 0.0)
                for M0 in (range(0, MS, 32) if not SKIP_MLP else []):
                    hs_g = []
                    for g in range(4):
                        ebase = M0 + 8 * g
                        emi = mp.tile([P, 8, 16], dt.bfloat16, tag="emi", name="emi")
                        inv = em[:, :, ebase:ebase + 8].rearrange("p k s -> p s k")
                        if g % 2 == 0:
                            nc.scalar.copy(out=emi[:], in_=inv)
                        else:
                            nc.vector.tensor_copy(out=emi[:], in_=inv)
                        embT = pst.tile([32, 512], dt.bfloat16, tag="embT", name="embT")
                        for q in range(4):
                            nc.tensor.transpose(out=embT[:, 128 * q:128 * (q + 1)],
                                                in_=emi[:, 2 * q:2 * q + 2, :],
                                                identity=identb[:])
                        embTs = mp.tile([32, 512], dt.bfloat16, tag="embTs", name="embTs")
                        if g % 2 == 0:
                            nc.vector.tensor_copy(out=embTs[:], in_=embT[:])
                        else:
                            nc.scalar.copy(out=embTs[:], in_=embT[:])
                        hp = psh.tile([P, 512], dt.float32, tag="hp", name="hp")
                        for q in range(4):
                            nc.tensor.matmul(
                                out=hp[:, 128 * q:128 * (q + 1)],
                                lhsT=w1pad[:],
                                rhs=embTs[:, 128 * q:128 * (q + 1)],
                                start=True, stop=True)
                        hs = mp.tile([P, 512], dt.bfloat16, tag="hs", name="hs")
                        if g % 2 == 0:
                            nc.scalar.activation(out=hs[:], in_=hp[:], func=Act.Relu)
                        else:
                            nc.vector.tensor_scalar_max(hs[:], hp[:], 0.0)
                        hs_g.append(hs)
                    wp = psw.tile([P, 512], dt.float32, tag="wp", name="wp")
                    if MM_SAFE:
                        nc.vector.memset(wp[32:128, :], 0.0)
                    for g in (range(4) if not SKIP_MM else []):
                        gg = 0 if MM_SAFE else g
                        nc.tensor.matmul(
                            out=wp[0:32, :] if MM_SAFE
                            else wp[32 * g:32 * g + 32, :],
                            lhsT=w2stack[:, 32 * gg:32 * gg + 32],
                            rhs=hs_g[g][:],
                            start=True, stop=True,
                            tile_position=None if MM_SAFE else (
                                (0, 32 * g) if USE_TILE_POS else None))
                    wTs = mp.tile([P, 512], dt.bfloat16, tag="wTs", name="wTs")
                    if (M0 // 32) % 2 == 0:
                        nc.scalar.copy(out=wTs[:], in_=wp[:])
                    else:
                        nc.vector.tensor_copy(out=wTs[:], in_=wp[:])
                    for q in range(4):
                        wt = pst.tile([P, P], dt.bfloat16, tag="wt", name="wt")
                        nc.tensor.transpose(out=wt[:],
                                            in_=wTs[:, 128 * q:128 * q + 128],
                                            identity=identb[:])
                        # out[p, jj, M0+8g+2q+c] = wt[p, 32g+2jj+c]
                        inv = wt.rearrange("p (g j c) -> p j g c", g=4, j=16, c=2)
                        inv = inv[:, 0:5, :, :]
                        base = M0 + 2 * q
                        outv = ws[:, :, base:base + 32]
                        outv = outv.rearrange("p w (g r) -> p w g r", g=4, r=8)
                        outv = outv[:, :, :, 0:2]
                        if q % 2 == 0:
                            nc.vector.tensor_copy(out=outv, in_=inv)
                        else:
                            nc.scalar.copy(out=outv, in_=inv)

                # D: tensor product on planes
                if SKIP_D:
                    if mi == 0:
                        for i in range(4):
                            nc.vector.memset(g8[i][:], 0.0)
                    continue
                x0 = rowfs[:, 0, off:off + MS]
                x1 = rowfs[:, 1, off:off + MS]
                x2 = rowfs[:, 2, off:off + MS]
                x3 = rowfs[:, 3, off:off + MS]
                # e3nn (y, z, x) order
                up1 = ufull[:, 1, off:off + MS]
                up2 = ufull[:, 2, off:off + MS]
                up3 = ufull[:, 0, off:off + MS]
                w0 = ws[:, 0, :MS]
                w1_ = ws[:, 1, :MS]
                w2_ = ws[:, 2, :MS]
                w3 = ws[:, 3, :MS]
                w4 = ws[:, 4, :MS]

                def B(tag):
                    t = cp.tile([P, MEGA], dt.bfloat16, tag=tag, name=tag)
                    return t[:, :MS]

                V = nc.vector
                G = nc.gpsimd if USE_GP else nc.vector
                s1, s2, s3 = B("s1"), B("s2"), B("s3")
                G.tensor_tensor(out=s1, in0=x1, in1=up1, op=Alu.mult)
                G.tensor_tensor(out=s2, in0=x2, in1=up2, op=Alu.mult)
                V.tensor_tensor(out=s3, in0=x3, in1=up3, op=Alu.mult)
                dot = B("dot")
                G.tensor_tensor(out=dot, in0=s1, in1=s2, op=Alu.add)
                V.tensor_tensor(out=dot, in0=dot, in1=s3, op=Alu.add)
                ca, cb = B("ca"), B("cb")
                cr1, cr2, cr3 = B("cr1"), B("cr2"), B("cr3")
                G.tensor_tensor(out=ca, in0=x2, in1=up3, op=Alu.mult)
                V.tensor_tensor(out=cb, in0=x3, in1=up2, op=Alu.mult)
                G.tensor_tensor(out=cr1, in0=ca, in1=cb, op=Alu.subtract)
                V.tensor_tensor(out=ca, in0=x3, in1=up1, op=Alu.mult)
                G.tensor_tensor(out=cb, in0=x1, in1=up3, op=Alu.mult)
                V.tensor_tensor(out=cr2, in0=ca, in1=cb, op=Alu.subtract)
                G.tensor_tensor(out=ca, in0=x1, in1=up2, op=Alu.mult)
                V.tensor_tensor(out=cb, in0=x2, in1=up1, op=Alu.mult)
                G.tensor_tensor(out=cr3, in0=ca, in1=cb, op=Alu.subtract)

                o = [B(f"o{i}") for i in range(4)]
                tmp = B("tmp")
                G.tensor_tensor(out=o[0], in0=w0, in1=x0, op=Alu.mult)
                V.tensor_tensor(out=tmp, in0=w3, in1=dot, op=Alu.mult)
                G.tensor_tensor(out=o[0], in0=o[0], in1=tmp, op=Alu.add)
                t1 = B("t1")
                V.tensor_tensor(out=t1, in0=w1_, in1=x0, op=Alu.mult)
                for i, (upc, xc, crc) in enumerate(
                        ((up1, x1, cr1), (up2, x2, cr2), (up3, x3, cr3))):
                    V.tensor_tensor(out=o[i + 1], in0=t1, in1=upc, op=Alu.mult)
                    G.tensor_tensor(out=tmp, in0=w2_, in1=xc, op=Alu.mult)
                    V.tensor_tensor(out=o[i + 1], in0=o[i + 1], in1=tmp, op=Alu.add)
                    V.tensor_tensor(out=tmp, in0=w4, in1=crc, op=Alu.mult)
                    V.tensor_tensor(out=o[i + 1], in0=o[i + 1], in1=tmp, op=Alu.add)

                for i in range(4):
                    nc.vector.tensor_reduce(
                        out=g8[i][:, off // 4:(off + MS) // 4],
                        in_=o[i].rearrange("p (a b) -> p a b", b=4),
                        op=Alu.add,
                        axis=mybir.AxisListType.X,
                    )

            # ---- per-class final reduction [P, NN] x4 ----
            nsum = pp.tile([P, NN, 4], dt.float32, name="nsum")
            foff8 = 0
            noff = 0
            for (k, nk) in class_list:
                k8 = k // 4
                for i in range(4):
                    nc.vector.tensor_reduce(
                        out=nsum[:, noff:noff + nk, i],
                        in_=g8[i][:, foff8:foff8 + nk * k8].rearrange(
                            "p (n g) -> p n g", g=k8),
                        op=Alu.add,
                        axis=mybir.AxisListType.X,
                    )
                foff8 += nk * k8
                noff += nk

            nc.sync.dma_start(out=yout[:], in_=nsum[:])

    nc.finalize()
    return nc


def kernel(f_1, pos, W1, W2, edge_index):
    import ml_dtypes
    f_1 = np.ascontiguousarray(f_1, np.float32)
    pos = np.ascontiguousarray(pos, np.float32)
    W1 = np.ascontiguousarray(W1, np.float32)
    W2 = np.ascontiguousarray(W2, np.float32)
    ei = np.asarray(edge_index).astype(np.int64)

    class_list, NN, F, row_slots, col_slot, node_lid = _build_layout(ei)
    nc = _build_program(class_list, NN, F)

    pos_aug = np.zeros((N_NODES + 1, 3), np.float32)
    pos_aug[:N_NODES] = pos
    f1_aug = np.zeros((N_NODES + 1, 4), np.float32)
    f1_aug[:N_NODES] = f_1

    in_maps = []
    for c in range(N_CORES):
        rposc = pos_aug[row_slots[c]].transpose(0, 2, 1)   # [P, 3, F]
        cposc = pos_aug[col_slot[c]].transpose(0, 2, 1)    # [P, 3, F]
        rfc = f1_aug[row_slots[c]].transpose(0, 2, 1)      # [P, 4, F]
        in_maps.append({
            "W1": W1, "W2": W2,
            "rowpos": np.ascontiguousarray(rposc),
            "colpos": np.ascontiguousarray(cposc),
            "rowf": np.ascontiguousarray(rfc),
        })
    trace = os.environ.get("KERNEL_TRACE", "0") == "1"
    if trace:
        _install_ntff_shim()
    res = run_bass_kernel_spmd(nc, in_maps, list(range(N_CORES)), trace=trace)
    global LAST_EXEC_NS
    LAST_EXEC_NS = res.exec_time_ns
    out = np.zeros((N_NODES, 4), np.float32)
    for c in range(N_CORES):
        yb = np.asarray(res.results[c]["yout"]).reshape(P, NN, 4)
        lid = node_lid[c]
        valid = lid < NPC
        out[c * NPC + lid[valid]] = yb[valid]
    return out.astype(np.float32)


if __name__ == "__main__":
    import reference
    inputs = {k: np.asarray(v) for k, v in reference.setup_inputs().items()}
    out = kernel(**inputs)
    print("kernel out", out.shape, out.dtype)
